# revision 18
# baseline (speedup 1.0000x reference)
"""Trainium2 Bass kernel for the quantized BasicBlock (conv3x3/s2 + fakequant + conv3x3/s1 + fakequant).

Sharding: data-parallel over batch across 8 cores (8 images each), weights replicated.

Device math (per core, B=8):
  conv1: implicit GEMM in fp16 (x rounded to fp16 on host, int8 weights exact in
         fp16), 9 taps x 2 ci-blocks, fp32 PSUM accum. Stride-2 is handled by a
         host-side phase split: x is scattered into 2x2 parity planes, zero-padded
         to 15x15, so each tap reads a stride-1 14x14 window of one plane.
  act1:  v = P1*(s_w1/s_a1) + bq1/s_a1 on the ACT engine; y = clip(rne(v), -128, 127)
         via the fp32 magic-number trick on the DVE; y stored as int-valued fp16
         into zero-padded row-parity planes (even rows / odd rows separately, so
         the winograd row transform reads long contiguous runs).
  conv2: 1-D Winograd F(2,3) along rows x direct conv along columns.
         Row transform Bt: t0=E[ty]-E[ty+1], t1=O[ty]+E[ty+1], t2=E[ty+1]-O[ty],
         t3=O[ty]-O[ty+1] (DVE, exact ints in fp16). GEMM: per output row-pair
         coefficient r', accumulate 3 column taps x 4 ci-blocks into one PSUM
         bank (fp16 weights G@g are half-integers <= 190.5, exact). Output
         transform At: u0=M0+M1+M2, u1=M1-M2-M3 (ACT evacuates M1, DVE combines).
  act2:  v2 = u*(s_a1*s_w2/s_a2) + bq2/s_a2 (ACT); clip(rne(v2)) (DVE, interleaving
         the row pairs back into (img, 14, 14) layout); * s_a2 (ACT); one
         contiguous DMA out per (co-block, image-half).
"""
import os
import sys
from contextlib import ExitStack

import numpy as np
import ml_dtypes

for _p in ("/opt/trn_rl_repo",):
    if _p not in sys.path and os.path.isdir(_p):
        sys.path.insert(0, _p)

import concourse.bacc as bacc
import concourse.tile as tile
import concourse.mybir as mybir
from concourse.bass_utils import run_bass_kernel_spmd

BF16 = ml_dtypes.bfloat16
N_CORES = 8
B_PER = 8           # images per core
MAGIC = float(np.float32(1.5 * 2 ** 23))   # fp32 RNE rounding magic
Alu = mybir.AluOpType
Act = mybir.ActivationFunctionType
dt = mybir.dt

# tap index k in {0,1,2} -> (parity s, window start offset) for the phase planes
_TAP = {0: (1, 0), 1: (0, 1), 2: (1, 1)}


def _phase_planes(x):
    """(B, C, 28, 28) f32 -> (B, C, 2, 2, 15, 15): plane[sr][sc][q+1][p+1] = x[2q+sr][2p+sc]."""
    B, C = x.shape[:2]
    out = np.zeros((B, C, 2, 2, 15, 15), np.float32)
    for sr in (0, 1):
        for sc in (0, 1):
            out[:, :, sr, sc, 1:15, 1:15] = x[:, :, sr::2, sc::2]
    return out


def _quant_weights(w):
    """Per-tensor int8 narrow-range fake quant; returns (int-valued f32 weights, scale)."""
    s = np.float32(np.max(np.abs(w))) / np.float32(127.0)
    wq = np.clip(np.round(w / s), -127, 127).astype(np.float32)
    return wq, s


_skip_ldw = [False]
_orig_InstMatmult = mybir.InstMatmult


def _patched_InstMatmult(*a, **kw):
    if _skip_ldw[0]:
        kw.setdefault("ldweights", False)
    return _orig_InstMatmult(*a, **kw)


def build_program(scale1, scale2, out_scale):
    """Build the (per-core SPMD) Bass program with the given fp32 immediates."""
    nc = bacc.Bacc("TRN2", target_bir_lowering=False, debug=False,
                   num_devices=N_CORES)

    mybir.InstMatmult = _patched_InstMatmult
    try:
        return _build_body(nc, scale1, scale2, out_scale)
    finally:
        mybir.InstMatmult = _orig_InstMatmult


def _build_body(nc, scale1, scale2, out_scale):
    NT = 4

    x_d = nc.dram_tensor("xp", (128, 2, 4, B_PER, 15, 15), dt.float16, kind="ExternalInput")
    w1_d = nc.dram_tensor("w1", (2, 128, 9, 4, 128), dt.int8, kind="ExternalInput")
    w2w_d = nc.dram_tensor("w2w", (4, 4, 128, 4, 3, 128), dt.float16, kind="ExternalInput")
    b1_d = nc.dram_tensor("b1", (128, 4), dt.float32, kind="ExternalInput")
    b2_d = nc.dram_tensor("b2", (128, 4), dt.float32, kind="ExternalInput")
    out_d = nc.dram_tensor("out", (512, B_PER, 14, 14), dt.float32, kind="ExternalOutput")

    def mm(out_ap, w_ap, rhs, start, stop, reuse):
        _skip_ldw[0] = reuse
        try:
            nc.tensor.matmul(out_ap, w_ap, rhs, start=start, stop=stop)
        finally:
            _skip_ldw[0] = False

    with tile.TileContext(nc) as tc, ExitStack() as ctx:
        const = ctx.enter_context(tc.tile_pool(name="const", bufs=1))
        psum = ctx.enter_context(tc.tile_pool(name="psum", bufs=8, space="PSUM"))
        tmp = ctx.enter_context(tc.tile_pool(name="tmp", bufs=2))
        outp = ctx.enter_context(tc.tile_pool(name="outp", bufs=2))
        w1cp = ctx.enter_context(tc.tile_pool(name="w1cp", bufs=2))
        w2pool = ctx.enter_context(tc.tile_pool(name="w2pool", bufs=6))
        upool = ctx.enter_context(tc.tile_pool(name="upool", bufs=2))

        # --- persistent SBUF ---
        x_t = [const.tile([128, 2, B_PER, 15, 15], dt.float16, tag=f"xp{pl}", name=f"xp{pl}")
               for pl in range(4)]
        w1i_t = [const.tile([128, 9, 4, 128], dt.int8, tag=f"w1i{b}", name=f"w1i{b}") for b in range(2)]
        # act1 as row-parity planes: E holds padded rows 0,2..14, O holds 1,3..15
        acte_t = [const.tile([128, B_PER, 8, 16], dt.float16, tag=f"ae{b}", name=f"ae{b}")
                  for b in range(4)]
        acto_t = [const.tile([128, B_PER, 8, 16], dt.float16, tag=f"ao{b}", name=f"ao{b}")
                  for b in range(4)]
        # winograd row-transformed activations, per ci-block
        tr_t = [const.tile([128, 4, B_PER, 7, 16], dt.float16, tag=f"tr{b}", name=f"tr{b}")
                for b in range(4)]
        b1_t = const.tile([128, 4], dt.float32, tag="b1")
        b2_t = const.tile([128, 4], dt.float32, tag="b2")
        wz = const.tile([128, 256], dt.bfloat16, tag="wz")

        nc.vector.memset(wz[:], 0.0)
        for b in range(4):
            nc.gpsimd.memset(acte_t[b][:], 0.0)
            nc.gpsimd.memset(acto_t[b][:], 0.0)

        def load(dst, src):
            nc.sync.dma_start(out=dst, in_=src)

        # w1 first (small, needed by the first matmul), then whole planes in
        # first-use order, all on the Sync queue
        for b in range(2):
            load(w1i_t[b][:], w1_d[b])
        load(x_t[3][:], x_d[:, :, 3])
        load(b1_t[:], b1_d[:])
        load(x_t[2][:], x_d[:, :, 2])
        load(x_t[1][:], x_d[:, :, 1])
        load(x_t[0][:], x_d[:, :, 0])
        load(b2_t[:], b2_d[:])

        # PE warm-up during the input-DMA window
        wps = psum.tile([128, 512], dt.float32, tag="ps", name="warmps")
        for i in range(58):
            nc.tensor.matmul(wps[:, 0:256], wz[:, 0:128], wz[:, 0:256],
                             start=True, stop=True)

        # --- conv1 + act1 (quant into row-parity planes) ---
        def c1_tap(t9):
            ky, kx = divmod(t9, 3)
            sr, r0 = _TAP[ky]
            sc_, c0 = _TAP[kx]
            return sr * 2 + sc_, r0, c0

        def conv1_group(w1c, t9, b, ps_list, nts):
            pl, r0, c0 = c1_tap(t9)
            w_ap = w1c[:, b, t9, :]
            first = True
            for i, nt in enumerate(nts):
                rhs = x_t[pl][:, b, 2 * nt:2 * nt + 2, r0:r0 + 14, c0:c0 + 14]
                mm(ps_list[i][:, 0:392], w_ap, rhs,
                   start=(t9 == 0 and b == 0),
                   stop=(t9 == 8 and b == 1),
                   reuse=not first)
                first = False

        def quant_act1(cb, nt, ps):
            """act1 rows: conv1-out row r -> padded row r+1; split by parity."""
            tt = tmp.tile([128, 2, 14, 14], dt.float32, tag="tt392", name="tt")
            nc.scalar.activation(tt[:], ps[:, 0:392], Act.Identity,
                                 bias=b1_t[:, cb:cb + 1], scale=scale1)
            nc.vector.tensor_scalar(tt[:], tt[:], MAGIC, MAGIC + 127.0, op0=Alu.add, op1=Alu.min)
            # even padded rows 2,4..14  <- out rows 1,3..13
            nc.vector.tensor_scalar(
                acte_t[cb][:, 2 * nt:2 * nt + 2, 1:8, 1:15], tt[:, :, 1::2, :],
                MAGIC - 128.0, -MAGIC, op0=Alu.max, op1=Alu.add)
            # odd padded rows 1,3..13  <- out rows 0,2..12
            nc.vector.tensor_scalar(
                acto_t[cb][:, 2 * nt:2 * nt + 2, 0:7, 1:15], tt[:, :, 0::2, :],
                MAGIC - 128.0, -MAGIC, op0=Alu.max, op1=Alu.add)

        def row_transform(b):
            """tr[r'] over tiles ty: Bt row combos of parity planes (long runs)."""
            E0 = acte_t[b][:, :, 0:7, :]
            E1 = acte_t[b][:, :, 1:8, :]
            O0 = acto_t[b][:, :, 0:7, :]
            O1 = acto_t[b][:, :, 1:8, :]
            tr = tr_t[b]
            nc.vector.tensor_tensor(tr[:, 0], E0, E1, op=Alu.subtract)
            nc.vector.tensor_tensor(tr[:, 1], O0, E1, op=Alu.add)
            nc.vector.tensor_tensor(tr[:, 2], E1, O0, op=Alu.subtract)
            nc.vector.tensor_tensor(tr[:, 3], O0, O1, op=Alu.subtract)

        for cb in range(4):
            w1c = w1cp.tile([128, 2, 9, 128], dt.float16, tag="w1c", name="w1c")
            for b in range(2):
                nc.scalar.copy(w1c[:, b, :, :], w1i_t[b][:, :, cb, :])
            if cb == 0:
                # tap-major: plane demand spread over the whole group to match
                # the DMA delivery ramp
                ps_n = [psum.tile([128, 512], dt.float32, tag="ps", name="ps")
                        for _ in range(NT)]
                for t9 in range(9):
                    for b in range(2):
                        conv1_group(w1c, t9, b, ps_n, range(NT))
                for nt in range(NT):
                    quant_act1(cb, nt, ps_n[nt])
            else:
                for half in range(2):
                    nts = [2 * half, 2 * half + 1]
                    ps_p = [psum.tile([128, 512], dt.float32, tag="ps", name="ps")
                            for _ in nts]
                    for t9 in range(9):
                        for b in range(2):
                            conv1_group(w1c, t9, b, ps_p, nts)
                    for i, nt in enumerate(nts):
                        quant_act1(cb, nt, ps_p[i])
            row_transform(cb)

        # --- conv2: 1-D winograd GEMM + output transform + act2 ---
        for cb in range(4):
            w2r = []
            for rp in range(4):
                wt_ = w2pool.tile([128, 4, 3, 128], dt.float16, tag="w2w", name="w2w")
                load(wt_[:], w2w_d[cb, rp])
                w2r.append(wt_)
            ot = outp.tile([128, B_PER, 14, 14], dt.float32, tag="ot", name="ot")

            def conv2_chunk(i0, ncnt):
                W = ncnt * 98
                ps4 = [psum.tile([128, 512], dt.float32, tag="ps", name="ps")
                       for _ in range(4)]
                for rp in range(4):
                    for b in range(4):
                        for kx in range(3):
                            rhs = tr_t[b][:, rp, i0:i0 + ncnt, :, kx:kx + 14]
                            mm(ps4[rp][:, 0:W], w2r[rp][:, b, kx, :], rhs,
                               start=(b == 0 and kx == 0),
                               stop=(b == 3 and kx == 2), reuse=False)
                # output transform At rows: u0 = M0+M1+M2, u1 = M1-M2-M3
                e1 = tmp.tile([128, 392], dt.float32, tag="e1", name="e1")
                nc.scalar.copy(e1[:, 0:W], ps4[1][:, 0:W])
                u = upool.tile([128, 2, 392], dt.float32, tag="u", name="u")
                nc.vector.tensor_tensor(u[:, 0, 0:W], e1[:, 0:W], ps4[0][:, 0:W], op=Alu.add)
                nc.vector.tensor_tensor(u[:, 0, 0:W], u[:, 0, 0:W], ps4[2][:, 0:W], op=Alu.add)
                nc.vector.tensor_tensor(u[:, 1, 0:W], e1[:, 0:W], ps4[2][:, 0:W], op=Alu.subtract)
                nc.vector.tensor_tensor(u[:, 1, 0:W], u[:, 1, 0:W], ps4[3][:, 0:W], op=Alu.subtract)
                # act2 quant; per-o chains pipeline across ACT/DVE, op3
                # interleaves row pairs into (img, 14, 14)
                tt = tmp.tile([128, 2, 392], dt.float32, tag="tt784", name="tt2")
                for o in range(2):
                    nc.scalar.activation(tt[:, o, 0:W], u[:, o, 0:W], Act.Identity,
                                         bias=b2_t[:, cb:cb + 1], scale=scale2)
                    nc.vector.tensor_scalar(tt[:, o, 0:W], tt[:, o, 0:W], MAGIC,
                                            MAGIC + 127.0, op0=Alu.add, op1=Alu.min)
                    nc.vector.tensor_scalar(
                        ot[:, i0:i0 + ncnt, o::2, :], tt[:, o, 0:W],
                        MAGIC - 128.0, -MAGIC, op0=Alu.max, op1=Alu.add)
                nc.scalar.mul(ot[:, i0:i0 + ncnt], ot[:, i0:i0 + ncnt], out_scale)
                nc.scalar.dma_start(out=out_d[cb * 128:(cb + 1) * 128, i0:i0 + ncnt],
                                    in_=ot[:, i0:i0 + ncnt])

            # last chunks shrink so the final epilogue chain is short
            chunks = ((0, 4), (4, 4)) if cb < 3 else ((0, 4), (4, 3), (7, 1))
            for i0, ncnt in chunks:
                conv2_chunk(i0, ncnt)

    _dedupe_ldweights(nc)
    nc.compile()
    return nc


def _dedupe_ldweights(nc):
    """Drop LDWEIGHTS whose stationary operand is identical to the previous
    one on the PE stream (only MATMULs in between)."""
    def sig_of(inst):
        a0 = inst.ins[0]
        try:
            return (a0.memref, a0.offset, str(a0.ap), str(a0.dtype))
        except Exception:
            return None

    removed = 0
    for blk in nc.main_func.blocks:
        last = None
        keep = []
        for inst in blk.instructions:
            tn = type(inst).__name__
            if inst.engine == mybir.EngineType.PE:
                if tn == "InstLdweights":
                    sig = sig_of(inst)
                    si = inst.sync_info
                    clean = si is None or (not si.on_wait and not si.on_update)
                    if sig is not None and sig == last and clean:
                        removed += 1
                        continue
                    last = sig
                elif tn != "InstMatmult":
                    last = None
            keep.append(inst)
        blk.instructions[:] = keep
    return removed


_G = np.array([[1.0, 0.0, 0.0], [0.5, 0.5, 0.5], [0.5, -0.5, 0.5], [0.0, 0.0, 1.0]],
              np.float32)


def prepare(x, w1, b1, w2, b2, in_scale, act1_scale, act2_scale):
    """Host-side prep: quantize weights, 1-D winograd-transform w2, build inputs."""
    x = np.asarray(x, np.float32)
    w1 = np.asarray(w1, np.float32)
    b1 = np.asarray(b1, np.float32)
    w2 = np.asarray(w2, np.float32)
    b2 = np.asarray(b2, np.float32)
    s_in = np.float32(np.asarray(in_scale).reshape(-1)[0])
    s_a1 = np.float32(np.asarray(act1_scale).reshape(-1)[0])
    s_a2 = np.float32(np.asarray(act2_scale).reshape(-1)[0])

    w1_int, s_w1 = _quant_weights(w1)
    w2_int, s_w2 = _quant_weights(w2)
    bq1 = np.clip(np.round(b1 / (s_in * s_w1)), -2.0 ** 31, 2.0 ** 31 - 1).astype(np.float32) * (s_in * s_w1)
    bq2 = np.clip(np.round(b2 / (s_a1 * s_w2)), -2.0 ** 31, 2.0 ** 31 - 1).astype(np.float32) * (s_a1 * s_w2)

    scale1 = float(np.float32(s_w1 / s_a1))
    scale2 = float(np.float32(s_a1 * s_w2 / s_a2))
    out_scale = float(s_a2)
    bias1 = np.ascontiguousarray((bq1 / s_a1).astype(np.float32).reshape(4, 128).T)  # (128, 4)
    bias2 = np.ascontiguousarray((bq2 / s_a2).astype(np.float32).reshape(4, 128).T)

    xp = _phase_planes(x)                                  # (64, 256, 2, 2, 15, 15)

    # conv1 weights: (512, 256, 3, 3) int -> (ci_blk 2, ci 128, tap 9, cb 4, co 128) int8
    t = w1_int.transpose(2, 3, 1, 0).reshape(9, 2, 128, 4, 128)
    w1_l = np.ascontiguousarray(t.transpose(1, 2, 0, 3, 4)).astype(np.int8)

    # conv2 1-D winograd weights: G @ g (rows) -> (cb, r', ci_p, b, kx, co_p) fp16
    ww = np.einsum('rk,oikl->oirl', _G, w2_int)            # (512co, 512ci, 4r, 3kx)
    ww = ww.reshape(4, 128, 4, 128, 4, 3)                  # (cb, co_p, b, ci_p, r, kx)
    ww = np.ascontiguousarray(ww.transpose(0, 4, 3, 2, 5, 1))  # (cb, r, ci_p, b, kx, co_p)
    w2w_l = ww.astype(np.float16)
    assert np.array_equal(w2w_l.astype(np.float32), ww), "w2w not exact in fp16"

    in_maps = []
    for c in range(N_CORES):
        sl = slice(c * B_PER, (c + 1) * B_PER)
        m = {}
        a = xp[sl].transpose(1, 2, 3, 0, 4, 5).reshape(2, 128, 4, B_PER, 15, 15)
        m["xp"] = np.ascontiguousarray(a.transpose(1, 0, 2, 3, 4, 5)).astype(np.float16)
        m["w1"] = w1_l
        m["w2w"] = w2w_l
        m["b1"] = bias1
        m["b2"] = bias2
        in_maps.append(m)
    return (scale1, scale2, out_scale), in_maps


def gather_out(results):
    """Per-core (512, 8, 14, 14) outputs -> full (64, 512, 14, 14)."""
    out = np.empty((N_CORES * B_PER, 512, 14, 14), np.float32)
    for c, r in enumerate(results):
        o = np.asarray(r["out"])                           # (512, 8, 14, 14)
        out[c * B_PER:(c + 1) * B_PER] = o.transpose(1, 0, 2, 3)
    return out


_cache = {}


def kernel(x, w1, b1, w2, b2, in_scale, act1_scale, act2_scale):
    imms, in_maps = prepare(x, w1, b1, w2, b2, in_scale, act1_scale, act2_scale)
    if imms not in _cache:
        _cache[imms] = build_program(*imms)
    nc = _cache[imms]
    res = run_bass_kernel_spmd(nc, in_maps, list(range(N_CORES)))
    return gather_out(res.results)


# revision 19
# speedup vs baseline: 1.1856x; 1.1856x over previous
"""Trainium2 Bass kernel for the quantized BasicBlock (conv3x3/s2 + fakequant + conv3x3/s1 + fakequant).

Sharding: data-parallel over batch across 8 cores (8 images each), weights replicated.

Device math (per core, B=8):
  conv1: implicit GEMM in fp16 (x rounded to fp16 on host, int8 weights exact in
         fp16), 9 taps x 2 ci-blocks, fp32 PSUM accum. Stride-2 is handled by a
         host-side phase split: x is scattered into 2x2 parity planes, zero-padded
         to 15x15, so each tap reads a stride-1 14x14 window of one plane.
  act1:  v = P1*(s_w1/s_a1) + bq1/s_a1 on the ACT engine; y = clip(rne(v), -128, 127)
         via the fp32 magic-number trick on the DVE; y stored as int-valued fp16
         into zero-padded row-parity planes (even rows / odd rows separately, so
         the winograd row transform reads long contiguous runs).
  conv2: 1-D Winograd F(2,3) along rows x direct conv along columns.
         Row transform Bt: t0=E[ty]-E[ty+1], t1=O[ty]+E[ty+1], t2=E[ty+1]-O[ty],
         t3=O[ty]-O[ty+1] (DVE, exact ints in fp16). GEMM: per output row-pair
         coefficient r', accumulate 3 column taps x 4 ci-blocks into one PSUM
         bank (fp16 weights G@g are half-integers <= 190.5, exact). Output
         transform At: u0=M0+M1+M2, u1=M1-M2-M3 (ACT evacuates M1, DVE combines).
  act2:  v2 = u*(s_a1*s_w2/s_a2) + bq2/s_a2 (ACT); clip(rne(v2)) (DVE, interleaving
         the row pairs back into (img, 14, 14) layout); * s_a2 (ACT); one
         contiguous DMA out per (co-block, image-half).
"""
import os
import sys
from contextlib import ExitStack

import numpy as np
import ml_dtypes

for _p in ("/opt/trn_rl_repo",):
    if _p not in sys.path and os.path.isdir(_p):
        sys.path.insert(0, _p)

import concourse.bacc as bacc
import concourse.tile as tile
import concourse.mybir as mybir
from concourse.bass_utils import run_bass_kernel_spmd

BF16 = ml_dtypes.bfloat16
N_CORES = 8
B_PER = 8           # images per core
MAGIC = float(np.float32(1.5 * 2 ** 23))   # fp32 RNE rounding magic
Alu = mybir.AluOpType
Act = mybir.ActivationFunctionType
dt = mybir.dt

# tap index k in {0,1,2} -> (parity s, window start offset) for the phase planes
_TAP = {0: (1, 0), 1: (0, 1), 2: (1, 1)}


def _phase_planes(x):
    """(B, C, 28, 28) f32 -> (B, C, 2, 2, 15, 15): plane[sr][sc][q+1][p+1] = x[2q+sr][2p+sc]."""
    B, C = x.shape[:2]
    out = np.zeros((B, C, 2, 2, 15, 15), np.float32)
    for sr in (0, 1):
        for sc in (0, 1):
            out[:, :, sr, sc, 1:15, 1:15] = x[:, :, sr::2, sc::2]
    return out


def _quant_weights(w):
    """Per-tensor int8 narrow-range fake quant; returns (int-valued f32 weights, scale)."""
    s = np.float32(np.max(np.abs(w))) / np.float32(127.0)
    wq = np.clip(np.round(w / s), -127, 127).astype(np.float32)
    return wq, s


_skip_ldw = [False]
_orig_InstMatmult = mybir.InstMatmult


def _patched_InstMatmult(*a, **kw):
    if _skip_ldw[0]:
        kw.setdefault("ldweights", False)
    return _orig_InstMatmult(*a, **kw)


def build_program(scale1, scale2, out_scale):
    """Build the (per-core SPMD) Bass program with the given fp32 immediates."""
    nc = bacc.Bacc("TRN2", target_bir_lowering=False, debug=False,
                   num_devices=N_CORES)

    mybir.InstMatmult = _patched_InstMatmult
    try:
        return _build_body(nc, scale1, scale2, out_scale)
    finally:
        mybir.InstMatmult = _orig_InstMatmult


def _build_body(nc, scale1, scale2, out_scale):
    NT = 4

    x_d = nc.dram_tensor("xp", (128, 2, 4, B_PER, 15, 15), dt.float16, kind="ExternalInput")
    w1_d = nc.dram_tensor("w1", (2, 128, 9, 4, 128), dt.int8, kind="ExternalInput")
    w2w_d = nc.dram_tensor("w2w", (4, 4, 128, 4, 3, 128), dt.float16, kind="ExternalInput")
    b1_d = nc.dram_tensor("b1", (128, 4), dt.float32, kind="ExternalInput")
    b2_d = nc.dram_tensor("b2", (128, 4), dt.float32, kind="ExternalInput")
    out_d = nc.dram_tensor("out", (512, B_PER, 14, 14), dt.float32, kind="ExternalOutput")

    def mm(out_ap, w_ap, rhs, start, stop, reuse):
        _skip_ldw[0] = reuse
        try:
            nc.tensor.matmul(out_ap, w_ap, rhs, start=start, stop=stop)
        finally:
            _skip_ldw[0] = False

    with tile.TileContext(nc) as tc, ExitStack() as ctx:
        const = ctx.enter_context(tc.tile_pool(name="const", bufs=1))
        psum = ctx.enter_context(tc.tile_pool(name="psum", bufs=8, space="PSUM"))
        tmp = ctx.enter_context(tc.tile_pool(name="tmp", bufs=2))
        outp = ctx.enter_context(tc.tile_pool(name="outp", bufs=2))
        w1cp = ctx.enter_context(tc.tile_pool(name="w1cp", bufs=2))
        w2pool = ctx.enter_context(tc.tile_pool(name="w2pool", bufs=6))
        upool = ctx.enter_context(tc.tile_pool(name="upool", bufs=2))

        # --- persistent SBUF ---
        x_t = [const.tile([128, 2, B_PER, 15, 15], dt.float16, tag=f"xp{pl}", name=f"xp{pl}")
               for pl in range(4)]
        w1i_t = [const.tile([128, 9, 4, 128], dt.int8, tag=f"w1i{b}", name=f"w1i{b}") for b in range(2)]
        # act1 as row-parity planes: E holds padded rows 0,2..14, O holds 1,3..15
        acte_t = [const.tile([128, B_PER, 8, 16], dt.float16, tag=f"ae{b}", name=f"ae{b}")
                  for b in range(4)]
        acto_t = [const.tile([128, B_PER, 8, 16], dt.float16, tag=f"ao{b}", name=f"ao{b}")
                  for b in range(4)]
        # winograd row-transformed activations, per ci-block
        tr_t = [const.tile([128, 4, B_PER, 7, 16], dt.float16, tag=f"tr{b}", name=f"tr{b}")
                for b in range(4)]
        b1_t = const.tile([128, 4], dt.float32, tag="b1")
        b2_t = const.tile([128, 4], dt.float32, tag="b2")
        wz = const.tile([128, 256], dt.bfloat16, tag="wz")

        nc.vector.memset(wz[:], 0.0)
        for b in range(4):
            nc.gpsimd.memset(acte_t[b][:], 0.0)
            nc.gpsimd.memset(acto_t[b][:], 0.0)

        def load(dst, src):
            nc.sync.dma_start(out=dst, in_=src)

        # w1 first (small, needed by the first matmul), then whole planes in
        # first-use order, all on the Sync queue
        for b in range(2):
            load(w1i_t[b][:], w1_d[b])
        load(x_t[3][:], x_d[:, :, 3])
        load(b1_t[:], b1_d[:])
        load(x_t[2][:], x_d[:, :, 2])
        load(x_t[1][:], x_d[:, :, 1])
        load(x_t[0][:], x_d[:, :, 0])
        load(b2_t[:], b2_d[:])

        # PE warm-up during the input-DMA window
        wps = psum.tile([128, 512], dt.float32, tag="ps", name="warmps")
        for i in range(58):
            nc.tensor.matmul(wps[:, 0:256], wz[:, 0:128], wz[:, 0:256],
                             start=True, stop=True)

        # --- conv1 + act1 (quant into row-parity planes) ---
        def c1_tap(t9):
            ky, kx = divmod(t9, 3)
            sr, r0 = _TAP[ky]
            sc_, c0 = _TAP[kx]
            return sr * 2 + sc_, r0, c0

        def conv1_group(w1c, t9, b, ps_list, nts):
            pl, r0, c0 = c1_tap(t9)
            w_ap = w1c[:, b, t9, :]
            first = True
            for i, nt in enumerate(nts):
                rhs = x_t[pl][:, b, 2 * nt:2 * nt + 2, r0:r0 + 14, c0:c0 + 14]
                mm(ps_list[i][:, 0:392], w_ap, rhs,
                   start=(t9 == 0 and b == 0),
                   stop=(t9 == 8 and b == 1),
                   reuse=not first)
                first = False

        def quant_act1(cb, nt, ps):
            """act1 rows: conv1-out row r -> padded row r+1; split by parity."""
            tt = tmp.tile([128, 2, 14, 14], dt.float32, tag="tt392", name="tt")
            nc.scalar.activation(tt[:], ps[:, 0:392], Act.Identity,
                                 bias=b1_t[:, cb:cb + 1], scale=scale1)
            nc.vector.tensor_scalar(tt[:], tt[:], MAGIC, MAGIC + 127.0, op0=Alu.add, op1=Alu.min)
            # even padded rows 2,4..14  <- out rows 1,3..13
            nc.vector.tensor_scalar(
                acte_t[cb][:, 2 * nt:2 * nt + 2, 1:8, 1:15], tt[:, :, 1::2, :],
                MAGIC - 128.0, -MAGIC, op0=Alu.max, op1=Alu.add)
            # odd padded rows 1,3..13  <- out rows 0,2..12
            nc.vector.tensor_scalar(
                acto_t[cb][:, 2 * nt:2 * nt + 2, 0:7, 1:15], tt[:, :, 0::2, :],
                MAGIC - 128.0, -MAGIC, op0=Alu.max, op1=Alu.add)

        def row_transform(b):
            """tr[r'] over tiles ty: Bt row combos of parity planes (long runs)."""
            E0 = acte_t[b][:, :, 0:7, :]
            E1 = acte_t[b][:, :, 1:8, :]
            O0 = acto_t[b][:, :, 0:7, :]
            O1 = acto_t[b][:, :, 1:8, :]
            tr = tr_t[b]
            nc.vector.tensor_tensor(tr[:, 0], E0, E1, op=Alu.subtract)
            nc.vector.tensor_tensor(tr[:, 1], O0, E1, op=Alu.add)
            nc.vector.tensor_tensor(tr[:, 2], E1, O0, op=Alu.subtract)
            nc.vector.tensor_tensor(tr[:, 3], O0, O1, op=Alu.subtract)

        for cb in range(4):
            w1c = w1cp.tile([128, 2, 9, 128], dt.float16, tag="w1c", name="w1c")
            for b in range(2):
                nc.scalar.copy(w1c[:, b, :, :], w1i_t[b][:, :, cb, :])
            if cb == 0:
                # tap-major: plane demand spread over the whole group to match
                # the DMA delivery ramp
                ps_n = [psum.tile([128, 512], dt.float32, tag="ps", name="ps")
                        for _ in range(NT)]
                for t9 in range(9):
                    for b in range(2):
                        conv1_group(w1c, t9, b, ps_n, range(NT))
                for nt in range(NT):
                    quant_act1(cb, nt, ps_n[nt])
            else:
                for half in range(2):
                    nts = [2 * half, 2 * half + 1]
                    ps_p = [psum.tile([128, 512], dt.float32, tag="ps", name="ps")
                            for _ in nts]
                    for t9 in range(9):
                        for b in range(2):
                            conv1_group(w1c, t9, b, ps_p, nts)
                    for i, nt in enumerate(nts):
                        quant_act1(cb, nt, ps_p[i])
            row_transform(cb)

        # --- conv2: 1-D winograd GEMM + output transform + act2 ---
        for cb in range(4):
            w2r = []
            for rp in range(4):
                wt_ = w2pool.tile([128, 4, 3, 128], dt.float16, tag="w2w", name="w2w")
                load(wt_[:], w2w_d[cb, rp])
                w2r.append(wt_)
            ot = outp.tile([128, B_PER, 14, 14], dt.float32, tag="ot", name="ot")

            def conv2_chunk(i0, ncnt):
                W = ncnt * 98
                ps4 = [psum.tile([128, 512], dt.float32, tag="ps", name="ps")
                       for _ in range(4)]
                for rp in range(4):
                    for b in range(4):
                        for kx in range(3):
                            rhs = tr_t[b][:, rp, i0:i0 + ncnt, :, kx:kx + 14]
                            mm(ps4[rp][:, 0:W], w2r[rp][:, b, kx, :], rhs,
                               start=(b == 0 and kx == 0),
                               stop=(b == 3 and kx == 2), reuse=False)
                # output transform At rows: u0 = M0+M1+M2, u1 = M1-M2-M3
                e1 = tmp.tile([128, 392], dt.float32, tag="e1", name="e1")
                nc.scalar.copy(e1[:, 0:W], ps4[1][:, 0:W])
                u = upool.tile([128, 2, 392], dt.float32, tag="u", name="u")
                nc.vector.tensor_tensor(u[:, 0, 0:W], e1[:, 0:W], ps4[0][:, 0:W], op=Alu.add)
                nc.vector.tensor_tensor(u[:, 0, 0:W], u[:, 0, 0:W], ps4[2][:, 0:W], op=Alu.add)
                nc.vector.tensor_tensor(u[:, 1, 0:W], e1[:, 0:W], ps4[2][:, 0:W], op=Alu.subtract)
                nc.vector.tensor_tensor(u[:, 1, 0:W], u[:, 1, 0:W], ps4[3][:, 0:W], op=Alu.subtract)
                # act2 quant; op3 interleaves row pairs into (img, 14, 14)
                tt = tmp.tile([128, 2, 392], dt.float32, tag="tt784", name="tt2")
                nc.scalar.activation(tt[:, :, 0:W], u[:, :, 0:W], Act.Identity,
                                     bias=b2_t[:, cb:cb + 1], scale=scale2)
                nc.vector.tensor_scalar(tt[:, :, 0:W], tt[:, :, 0:W], MAGIC, MAGIC + 127.0,
                                        op0=Alu.add, op1=Alu.min)
                for o in range(2):
                    nc.vector.tensor_scalar(
                        ot[:, i0:i0 + ncnt, o::2, :], tt[:, o, 0:W],
                        MAGIC - 128.0, -MAGIC, op0=Alu.max, op1=Alu.add)
                nc.scalar.mul(ot[:, i0:i0 + ncnt], ot[:, i0:i0 + ncnt], out_scale)
                nc.scalar.dma_start(out=out_d[cb * 128:(cb + 1) * 128, i0:i0 + ncnt],
                                    in_=ot[:, i0:i0 + ncnt])

            # last chunks shrink so the final epilogue chain is short
            chunks = ((0, 4), (4, 4)) if cb < 3 else ((0, 4), (4, 2), (6, 2))
            for i0, ncnt in chunks:
                conv2_chunk(i0, ncnt)

    _dedupe_ldweights(nc)
    nc.compile()
    return nc


def _dedupe_ldweights(nc):
    """Drop LDWEIGHTS whose stationary operand is identical to the previous
    one on the PE stream (only MATMULs in between)."""
    def sig_of(inst):
        a0 = inst.ins[0]
        try:
            return (a0.memref, a0.offset, str(a0.ap), str(a0.dtype))
        except Exception:
            return None

    removed = 0
    for blk in nc.main_func.blocks:
        last = None
        keep = []
        for inst in blk.instructions:
            tn = type(inst).__name__
            if inst.engine == mybir.EngineType.PE:
                if tn == "InstLdweights":
                    sig = sig_of(inst)
                    si = inst.sync_info
                    clean = si is None or (not si.on_wait and not si.on_update)
                    if sig is not None and sig == last and clean:
                        removed += 1
                        continue
                    last = sig
                elif tn != "InstMatmult":
                    last = None
            keep.append(inst)
        blk.instructions[:] = keep
    return removed


_G = np.array([[1.0, 0.0, 0.0], [0.5, 0.5, 0.5], [0.5, -0.5, 0.5], [0.0, 0.0, 1.0]],
              np.float32)


def prepare(x, w1, b1, w2, b2, in_scale, act1_scale, act2_scale):
    """Host-side prep: quantize weights, 1-D winograd-transform w2, build inputs."""
    x = np.asarray(x, np.float32)
    w1 = np.asarray(w1, np.float32)
    b1 = np.asarray(b1, np.float32)
    w2 = np.asarray(w2, np.float32)
    b2 = np.asarray(b2, np.float32)
    s_in = np.float32(np.asarray(in_scale).reshape(-1)[0])
    s_a1 = np.float32(np.asarray(act1_scale).reshape(-1)[0])
    s_a2 = np.float32(np.asarray(act2_scale).reshape(-1)[0])

    w1_int, s_w1 = _quant_weights(w1)
    w2_int, s_w2 = _quant_weights(w2)
    bq1 = np.clip(np.round(b1 / (s_in * s_w1)), -2.0 ** 31, 2.0 ** 31 - 1).astype(np.float32) * (s_in * s_w1)
    bq2 = np.clip(np.round(b2 / (s_a1 * s_w2)), -2.0 ** 31, 2.0 ** 31 - 1).astype(np.float32) * (s_a1 * s_w2)

    scale1 = float(np.float32(s_w1 / s_a1))
    scale2 = float(np.float32(s_a1 * s_w2 / s_a2))
    out_scale = float(s_a2)
    bias1 = np.ascontiguousarray((bq1 / s_a1).astype(np.float32).reshape(4, 128).T)  # (128, 4)
    bias2 = np.ascontiguousarray((bq2 / s_a2).astype(np.float32).reshape(4, 128).T)

    xp = _phase_planes(x)                                  # (64, 256, 2, 2, 15, 15)

    # conv1 weights: (512, 256, 3, 3) int -> (ci_blk 2, ci 128, tap 9, cb 4, co 128) int8
    t = w1_int.transpose(2, 3, 1, 0).reshape(9, 2, 128, 4, 128)
    w1_l = np.ascontiguousarray(t.transpose(1, 2, 0, 3, 4)).astype(np.int8)

    # conv2 1-D winograd weights: G @ g (rows) -> (cb, r', ci_p, b, kx, co_p) fp16
    ww = np.einsum('rk,oikl->oirl', _G, w2_int)            # (512co, 512ci, 4r, 3kx)
    ww = ww.reshape(4, 128, 4, 128, 4, 3)                  # (cb, co_p, b, ci_p, r, kx)
    ww = np.ascontiguousarray(ww.transpose(0, 4, 3, 2, 5, 1))  # (cb, r, ci_p, b, kx, co_p)
    w2w_l = ww.astype(np.float16)
    assert np.array_equal(w2w_l.astype(np.float32), ww), "w2w not exact in fp16"

    in_maps = []
    for c in range(N_CORES):
        sl = slice(c * B_PER, (c + 1) * B_PER)
        m = {}
        a = xp[sl].transpose(1, 2, 3, 0, 4, 5).reshape(2, 128, 4, B_PER, 15, 15)
        m["xp"] = np.ascontiguousarray(a.transpose(1, 0, 2, 3, 4, 5)).astype(np.float16)
        m["w1"] = w1_l
        m["w2w"] = w2w_l
        m["b1"] = bias1
        m["b2"] = bias2
        in_maps.append(m)
    return (scale1, scale2, out_scale), in_maps


def gather_out(results):
    """Per-core (512, 8, 14, 14) outputs -> full (64, 512, 14, 14)."""
    out = np.empty((N_CORES * B_PER, 512, 14, 14), np.float32)
    for c, r in enumerate(results):
        o = np.asarray(r["out"])                           # (512, 8, 14, 14)
        out[c * B_PER:(c + 1) * B_PER] = o.transpose(1, 0, 2, 3)
    return out


_cache = {}


def kernel(x, w1, b1, w2, b2, in_scale, act1_scale, act2_scale):
    imms, in_maps = prepare(x, w1, b1, w2, b2, in_scale, act1_scale, act2_scale)
    if imms not in _cache:
        _cache[imms] = build_program(*imms)
    nc = _cache[imms]
    res = run_bass_kernel_spmd(nc, in_maps, list(range(N_CORES)))
    return gather_out(res.results)
